# revision 2
# baseline (speedup 1.0000x reference)
"""GCNN layer (sinc-rotated filter bank + 7x7 conv) as a Bass/Tile kernel
on 8 Trainium2 NeuronCores.

Strategy: data-parallel over batch (16 images -> 2 per core). Each core:
  1. loads the full weight (128,128,7,7) and generates the 4 rotated filter
     banks on-device: per output channel co, PE-transpose W[co] ([ci,49] ->
     [49,ci]) then one fp32 matmul with Mcat = [I | M | M^2 | M^3] (49x196,
     host-precomputed sinc-rotation matrix) giving V[ci, (r,p,co)] -- the
     conv lhsT layout -- rounded to f32r on PSUM evacuation.
  2. zero-pads its 2 images into SBUF ([ci, img, 70, 70], f32r) and runs the
     conv as 49 shifted f32r matmuls per PSUM tile: out[co, 8rows x 64cols]
     accumulated over taps, 4 rotations x 2 images x 8 row-blocks = 64 tiles.
f32r streams the 128x512 moving operand at 1 cycle/row (4x faster than fp32)
at ~11-bit input mantissa, giving ~1.5e-4 relative output error.
"""

import numpy as np

import concourse.bacc as bacc
import concourse.mybir as mybir
from concourse.tile import TileContext
from concourse.bass_utils import run_bass_kernel_spmd

F32 = mybir.dt.float32
F32R = mybir.dt.float32r

B, CIN, H, W = 16, 128, 64, 64
COUT, KS = 128, 7
GROUP = 4
TAPS = KS * KS  # 49
N_CORES = 8
IMGS_PER_CORE = B // N_CORES  # 2
PAD = 3
HP = H + 2 * PAD  # 70
WP = W + 2 * PAD  # 70
ROWS_PER_TILE = 8  # 8 rows x 64 cols = 512 = one PSUM bank of fp32
N_TILES = H // ROWS_PER_TILE  # 8


def _mcat() -> np.ndarray:
    """[49, 4*49] = [I | M | M^2 | M^3], M the sinc-interp rotation matrix.

    Matches reference._sinc_int: new_x[..., p] = sum_ab x[..., a, b] *
    sinc(tx[p]-a) * sinc(ty[p]-b), i.e. right-multiplication by
    M[(a,b), p] = sx[a,p] * sy[b,p].
    """
    k = KS
    L = k * k
    th = np.float32(90.0)  # radians, faithful to the torch module
    c, s = np.float64(np.cos(th, dtype=np.float32)), np.float64(
        np.sin(th, dtype=np.float32)
    )
    A = np.array([[c, -s], [s, c]], dtype=np.float64)
    cx = np.arange(k, dtype=np.float64) - k // 2
    grid = np.stack(np.meshgrid(cx, cx, indexing="ij"), axis=-1).reshape(L, 2).T
    t = A @ grid
    tx = t[0] + k // 2 - 1
    ty = t[1] + k // 2
    old = np.arange(k, dtype=np.float64)
    sx = np.sinc(tx[None, :] - old[:, None])  # (k, L)
    sy = np.sinc(ty[None, :] - old[:, None])
    M = (sx[:, None, :] * sy[None, :, :]).reshape(L, L)
    blocks = [np.eye(L), M, M @ M, M @ M @ M]
    return np.concatenate(blocks, axis=1).astype(np.float32)  # (49, 196)


def _build():
    nc = bacc.Bacc("TRN2")
    x_in = nc.declare_dram_parameter(
        "x", [IMGS_PER_CORE, CIN, H, W], F32, isOutput=False
    )
    w_in = nc.declare_dram_parameter("w", [COUT, CIN, KS, KS], F32, isOutput=False)
    mcat_in = nc.declare_dram_parameter(
        "mcat", [TAPS, GROUP * TAPS], F32, isOutput=False
    )
    ident_in = nc.declare_dram_parameter("ident", [CIN, CIN], F32, isOutput=False)
    out = nc.declare_dram_parameter(
        "out", [IMGS_PER_CORE, GROUP * COUT, H, W], F32, isOutput=True
    )

    with TileContext(nc) as tc:
        with (
            tc.tile_pool(name="big", bufs=1) as big,
            tc.tile_pool(name="stage", bufs=2) as stage,
            tc.tile_pool(name="small", bufs=1) as small,
            tc.tile_pool(name="genl", bufs=3) as genl,
            tc.tile_pool(name="outp", bufs=4) as outp,
            tc.tile_pool(name="ps", bufs=1, space="PSUM") as ps,
        ):
            # ---- static SBUF buffers ----
            # weights in natural layout: [ci, (co, q)]
            wnat = big.tile([CIN, COUT * TAPS], F32)
            # generated filters, conv-ready: [ci, (r, p, co)]
            vbuf = big.tile([CIN, GROUP * TAPS * COUT], F32R)
            # padded images: [ci, (img, row, col)]
            xpad = big.tile([CIN, IMGS_PER_CORE * HP * WP], F32R)
            xpad_v = xpad.rearrange(
                "c (i h w) -> c i h w", i=IMGS_PER_CORE, h=HP, w=WP
            )
            mcat = small.tile([TAPS, GROUP * TAPS], F32)
            ident = small.tile([CIN, CIN], F32)

            # ---- loads ----
            # w[co, ci, q] -> wnat[ci, co*49+q]: per-partition 196B runs
            nc.sync.dma_start(
                out=wnat.rearrange("c (o q) -> c o q", o=COUT),
                in_=w_in[:, :, :, :]
                .rearrange("o c kh kw -> c o (kh kw)"),
            )
            nc.sync.dma_start(out=mcat, in_=mcat_in[:, :])
            nc.sync.dma_start(out=ident, in_=ident_in[:, :])

            # zero the padding borders (memset on f32r is rejected by the ISA
            # checker, and an f32-dtyped producer trips the f32r rounding
            # verifier -- so cast-copy zeros from a small f32 tile instead)
            zrow = small.tile([CIN, 3 * WP], F32)
            nc.vector.memset(zrow, 0.0)
            for img in range(IMGS_PER_CORE):
                nc.vector.tensor_copy(
                    xpad_v[:, img, 0:PAD, :],
                    zrow.rearrange("c (h w) -> c h w", h=PAD),
                )
                nc.vector.tensor_copy(
                    xpad_v[:, img, HP - PAD : HP, :],
                    zrow.rearrange("c (h w) -> c h w", h=PAD),
                )
                nc.vector.tensor_copy(
                    xpad_v[:, img, PAD : PAD + H, 0:PAD],
                    zrow[:, : H * PAD].rearrange("c (h w) -> c h w", h=H),
                )
                nc.vector.tensor_copy(
                    xpad_v[:, img, PAD : PAD + H, WP - PAD : WP],
                    zrow[:, : H * PAD].rearrange("c (h w) -> c h w", h=H),
                )
            CHUNK_ROWS = 16
            for img in range(IMGS_PER_CORE):
                for c0 in range(0, H, CHUNK_ROWS):
                    st = stage.tile([CIN, CHUNK_ROWS * W], F32, name="xstage")
                    nc.sync.dma_start(
                        out=st,
                        in_=x_in[img, :, c0 : c0 + CHUNK_ROWS, :].rearrange(
                            "c h w -> c (h w)"
                        ),
                    )
                    # rounds f32 -> f32r while writing into the padded layout
                    nc.vector.tensor_copy(
                        xpad_v[:, img, PAD + c0 : PAD + c0 + CHUNK_ROWS, PAD : PAD + W],
                        st.rearrange("c (h w) -> c h w", h=CHUNK_ROWS),
                    )

            # ---- filter generation ----
            wnat_v = wnat.rearrange("c (o q) -> c o q", o=COUT)
            vbuf_v = vbuf.rearrange(
                "c (r p o) -> c r p o", r=GROUP, p=TAPS, o=COUT
            )
            for co in range(COUT):
                trp = ps.tile(
                    [TAPS, CIN], mybir.dt.float32, name="trp", tag="tr", bufs=2
                )
                nc.tensor.transpose(trp, wnat_v[:, co, :], ident)
                wcoT = genl.tile([TAPS, CIN], F32, name="wcoT")
                nc.vector.tensor_copy(wcoT, trp)
                gps = ps.tile(
                    [CIN, GROUP * TAPS],
                    mybir.dt.float32,
                    name="gps",
                    tag="gen",
                    bufs=2,
                )
                # fp32 matmul (full precision): out[ci, (r,p)] for this co
                nc.tensor.matmul(gps, wcoT, mcat, start=True, stop=True)
                # strided evac into vbuf[:, :, :, co]; casts (rounds) to f32r
                nc.vector.tensor_copy(vbuf_v[:, :, :, co], gps)

            # ---- convolution ----
            out_v = out[:, :, :, :]  # [img, rco, y, x]
            for r in range(GROUP):
                for img in range(IMGS_PER_CORE):
                    for t in range(N_TILES):
                        acc = ps.tile(
                            [COUT, ROWS_PER_TILE * W],
                            mybir.dt.float32,
                            name="acc",
                            tag="conv",
                            bufs=4,
                        )
                        y0 = t * ROWS_PER_TILE
                        for p in range(TAPS):
                            dy, dx = p // KS, p % KS
                            rhs = xpad_v[
                                :, img, y0 + dy : y0 + dy + ROWS_PER_TILE,
                                dx : dx + W,
                            ]
                            nc.tensor.matmul(
                                acc,
                                vbuf_v[:, r, p, :],
                                rhs,
                                start=(p == 0),
                                stop=(p == TAPS - 1),
                            )
                        ot = outp.tile([COUT, ROWS_PER_TILE * W], F32, name="ot")
                        nc.vector.tensor_copy(ot, acc)
                        nc.sync.dma_start(
                            out=out_v[
                                img,
                                r * COUT : (r + 1) * COUT,
                                y0 : y0 + ROWS_PER_TILE,
                                :,
                            ].rearrange("o h w -> o (h w)"),
                            in_=ot,
                        )

    nc.finalize()
    return nc


_NC_CACHE = None


def _get_nc():
    global _NC_CACHE
    if _NC_CACHE is None:
        _NC_CACHE = _build()
    return _NC_CACHE


def kernel(
    input: np.ndarray,
    weight: np.ndarray,
    _trace: bool = False,
    _trace_cores=None,
    _result_holder: dict | None = None,
) -> np.ndarray:
    input = np.ascontiguousarray(np.asarray(input, dtype=np.float32))
    weight = np.ascontiguousarray(np.asarray(weight, dtype=np.float32))
    assert input.shape == (B, CIN, H, W), input.shape
    assert weight.shape == (COUT, CIN, KS, KS), weight.shape

    mcat = _mcat()
    ident = np.eye(CIN, dtype=np.float32)

    nc = _get_nc()
    in_maps = []
    for core in range(N_CORES):
        in_maps.append(
            {
                "x": input[core * IMGS_PER_CORE : (core + 1) * IMGS_PER_CORE],
                "w": weight,
                "mcat": mcat,
                "ident": ident,
            }
        )

    kwargs = {}
    if _trace:
        kwargs["trace"] = True
        if _trace_cores is not None:
            kwargs["trace_cores"] = _trace_cores

    res = run_bass_kernel_spmd(nc, in_maps, list(range(N_CORES)), **kwargs)
    if _result_holder is not None:
        _result_holder["res"] = res

    out = np.concatenate([res.results[c]["out"] for c in range(N_CORES)], axis=0)
    return out


# revision 6
# speedup vs baseline: 1.1085x; 1.1085x over previous
"""GCNN layer (sinc-rotated filter bank + 7x7 conv) as a Bass/Tile kernel
on 8 Trainium2 NeuronCores.

Strategy: data-parallel over batch (16 images -> 2 per core). Each core:
  1. loads the full weight (128,128,7,7), rounds it to f32r once, and
     generates the 3 rotated filter banks on-device: per output channel co,
     PE-transpose W[co] ([ci,49] -> [49,ci]) then one f32r matmul with
     Mcat = [M | M^2 | M^3] (49x147, host-precomputed sinc-rotation matrix,
     f32r-rounded on device). f32r products of pre-rounded operands are
     exact (11+11 mantissa bits accumulate in fp32), so the generated
     filters carry only the input-rounding error. Rotation 0 is the
     original weight: a bulk reshuffle copy, hidden under the conv by
     convolving rotation 0 last.
  2. zero-pads its 2 images into SBUF ([ci, img, 70, 70], f32r) and runs the
     conv as 49 shifted f32r matmuls per PSUM tile: out[co, 8rows x 64cols]
     accumulated over taps, 4 rotations x 2 images x 8 row-blocks = 64 tiles.
f32r streams the 128x512 moving operand at 1 cycle/row (4x faster than fp32)
at ~11-bit input mantissa, giving ~1.5e-4 relative output error.
"""

import numpy as np

import concourse.bacc as bacc
import concourse.mybir as mybir
from concourse.tile import TileContext
from concourse.bass_utils import run_bass_kernel_spmd

F32 = mybir.dt.float32
F32R = mybir.dt.float32r

B, CIN, H, W = 16, 128, 64, 64
COUT, KS = 128, 7
GROUP = 4
NROT = GROUP - 1  # generated rotations (r=1..3)
MCAT_N = NROT * 49 + 1  # padded to an even free dim (PSUM 8B granularity)
TAPS = KS * KS  # 49
N_CORES = 8
IMGS_PER_CORE = B // N_CORES  # 2
PAD = 3
HP = H + 2 * PAD  # 70
WP = W + 2 * PAD  # 70
ROWS_PER_TILE = 8  # 8 rows x 64 cols = 512 = one PSUM bank of fp32
N_TILES = H // ROWS_PER_TILE  # 8
COS_PER_GRP = 3  # gen psum batch: 3 * 147 * 4B = 1764B <= one 2KB bank


def _mcat() -> np.ndarray:
    """[49, 3*49] = [M | M^2 | M^3], M the sinc-interp rotation matrix.

    Matches reference._sinc_int: new_x[..., p] = sum_ab x[..., a, b] *
    sinc(tx[p]-a) * sinc(ty[p]-b), i.e. right-multiplication by
    M[(a,b), p] = sx[a,p] * sy[b,p].
    """
    k = KS
    L = k * k
    th = np.float32(90.0)  # radians, faithful to the torch module
    c, s = np.float64(np.cos(th, dtype=np.float32)), np.float64(
        np.sin(th, dtype=np.float32)
    )
    A = np.array([[c, -s], [s, c]], dtype=np.float64)
    cx = np.arange(k, dtype=np.float64) - k // 2
    grid = np.stack(np.meshgrid(cx, cx, indexing="ij"), axis=-1).reshape(L, 2).T
    t = A @ grid
    tx = t[0] + k // 2 - 1
    ty = t[1] + k // 2
    old = np.arange(k, dtype=np.float64)
    sx = np.sinc(tx[None, :] - old[:, None])  # (k, L)
    sy = np.sinc(ty[None, :] - old[:, None])
    M = (sx[:, None, :] * sy[None, :, :]).reshape(L, L)
    blocks = [M, M @ M, M @ M @ M, np.zeros((L, 1))]  # pad to even N
    return np.concatenate(blocks, axis=1).astype(np.float32)  # (49, 148)


def _build():
    nc = bacc.Bacc("TRN2")
    x_in = nc.declare_dram_parameter(
        "x", [IMGS_PER_CORE, CIN, H, W], F32, isOutput=False
    )
    w_in = nc.declare_dram_parameter("w", [COUT, CIN, KS, KS], F32, isOutput=False)
    mcat_in = nc.declare_dram_parameter(
        "mcat", [TAPS, MCAT_N], F32, isOutput=False
    )
    ident_in = nc.declare_dram_parameter("ident", [CIN, CIN], F32, isOutput=False)
    out = nc.declare_dram_parameter(
        "out", [IMGS_PER_CORE, GROUP * COUT, H, W], F32, isOutput=True
    )

    with TileContext(nc) as tc:
        with (
            tc.tile_pool(name="big", bufs=1) as big,
            tc.tile_pool(name="stage", bufs=2) as stage,
            tc.tile_pool(name="small", bufs=1) as small,
            tc.tile_pool(name="genl", bufs=3) as genl,
            tc.tile_pool(name="outp", bufs=4) as outp,
            tc.tile_pool(name="ps", bufs=1, space="PSUM") as ps,
        ):
            # ---- static SBUF buffers ----
            # f32r-rounded weights in natural layout: [ci, (co, q)]
            wr32 = big.tile([CIN, COUT * TAPS], F32R)
            wr32_v = wr32.rearrange("c (o q) -> c o q", o=COUT)
            # generated filters, conv-ready: [ci, (r, p, co)]
            vbuf = big.tile([CIN, GROUP * TAPS * COUT], F32R)
            vbuf_v = vbuf.rearrange(
                "c (r p o) -> c r p o", r=GROUP, p=TAPS, o=COUT
            )
            # padded images: [ci, (img, row, col)]
            xpad = big.tile([CIN, IMGS_PER_CORE * HP * WP], F32R)
            xpad_v = xpad.rearrange(
                "c (i h w) -> c i h w", i=IMGS_PER_CORE, h=HP, w=WP
            )
            mcat = small.tile([TAPS, MCAT_N], F32)
            mcat_r = small.tile([TAPS, MCAT_N], F32R)
            ident = small.tile([CIN, CIN], F32)
            ident_r = small.tile([CIN, CIN], F32R)

            # ---- loads ----
            nc.sync.dma_start(out=mcat, in_=mcat_in[:, :])
            nc.sync.dma_start(out=ident, in_=ident_in[:, :])
            nc.vector.tensor_copy(mcat_r, mcat)
            nc.vector.tensor_copy(ident_r, ident)

            # w[co, ci, q] -> wr32[ci, co*49+q] in 4 chunks of 32 cos so the
            # transposes can start as soon as the first chunk lands
            W_CHUNK = 32
            for c0 in range(0, COUT, W_CHUNK):
                wst = stage.tile([CIN, W_CHUNK * TAPS], F32, name="wstage")
                nc.sync.dma_start(
                    out=wst.rearrange("c (o q) -> c o q", o=W_CHUNK),
                    in_=w_in[c0 : c0 + W_CHUNK, :, :, :].rearrange(
                        "o c kh kw -> c o (kh kw)"
                    ),
                )
                nc.vector.tensor_copy(
                    wr32[:, c0 * TAPS : (c0 + W_CHUNK) * TAPS], wst
                )

            # zero the padding borders (memset on f32r is rejected by the ISA
            # checker, and an f32-dtyped producer trips the f32r rounding
            # verifier -- so cast-copy zeros from a small f32 tile instead)
            zrow = small.tile([CIN, 3 * WP], F32)
            nc.vector.memset(zrow, 0.0)
            for img in range(IMGS_PER_CORE):
                nc.vector.tensor_copy(
                    xpad_v[:, img, 0:PAD, :],
                    zrow.rearrange("c (h w) -> c h w", h=PAD),
                )
                nc.vector.tensor_copy(
                    xpad_v[:, img, HP - PAD : HP, :],
                    zrow.rearrange("c (h w) -> c h w", h=PAD),
                )
                nc.vector.tensor_copy(
                    xpad_v[:, img, PAD : PAD + H, 0:PAD],
                    zrow[:, : H * PAD].rearrange("c (h w) -> c h w", h=H),
                )
                nc.vector.tensor_copy(
                    xpad_v[:, img, PAD : PAD + H, WP - PAD : WP],
                    zrow[:, : H * PAD].rearrange("c (h w) -> c h w", h=H),
                )

            # stage + round the images into the padded buffer
            CHUNK_ROWS = 16
            for img in range(IMGS_PER_CORE):
                for c0 in range(0, H, CHUNK_ROWS):
                    st = stage.tile([CIN, CHUNK_ROWS * W], F32, name="xstage")
                    nc.sync.dma_start(
                        out=st,
                        in_=x_in[img, :, c0 : c0 + CHUNK_ROWS, :].rearrange(
                            "c h w -> c (h w)"
                        ),
                    )
                    nc.vector.tensor_copy(
                        xpad_v[:, img, PAD + c0 : PAD + c0 + CHUNK_ROWS, PAD : PAD + W],
                        st.rearrange("c (h w) -> c h w", h=CHUNK_ROWS),
                    )

            # ---- filter generation (rotations 1..3), 3 cos per psum bank ----
            co_groups = [
                (c0, min(COS_PER_GRP, COUT - c0))
                for c0 in range(0, COUT, COS_PER_GRP)
            ]
            for c0, ng in co_groups:
                trp = ps.tile(
                    [TAPS, COS_PER_GRP * CIN],
                    F32R,
                    name="trp",
                    tag="tr",
                    bufs=2,
                )
                for j in range(ng):
                    nc.tensor.transpose(
                        trp[:, j * CIN : (j + 1) * CIN],
                        wr32_v[:, c0 + j, :],
                        ident_r,
                    )
                wcoT = genl.tile([TAPS, COS_PER_GRP * CIN], F32R, name="wcoT")
                nc.vector.tensor_copy(
                    wcoT[:, : ng * CIN], trp[:, : ng * CIN]
                )
                gps = ps.tile(
                    [CIN, COS_PER_GRP * MCAT_N],
                    mybir.dt.float32,
                    name="gps",
                    tag="gen",
                    bufs=2,
                )
                for j in range(ng):
                    nc.tensor.matmul(
                        gps[:, j * MCAT_N : (j + 1) * MCAT_N],
                        wcoT[:, j * CIN : (j + 1) * CIN],
                        mcat_r,
                        start=True,
                        stop=True,
                    )
                # evac (rp-outer, co-inner): contiguous-ish writes into vbuf
                src = gps.rearrange(
                    "c (j rp) -> c j rp", j=COS_PER_GRP
                )[:, :, : NROT * TAPS].transpose([0, 2, 1])[:, :, :ng]
                dst = (
                    vbuf_v[:, 1:, :, c0 : c0 + ng]
                    .rearrange("c r p o -> c (r p) o")
                )
                nc.vector.tensor_copy(dst, src)

            # rotation 0 = the (rounded) original weight: pure reshuffle
            # [ci,(co,q)] -> [ci,(q->p, co)]; ~6.5k elems/partition. Runs on
            # DVE but is only needed by the conv's final r=0 quarter.
            nc.vector.tensor_copy(
                vbuf_v[:, 0, :, :],
                wr32_v.transpose([0, 2, 1]),
            )

            # ---- convolution (rotation 0 last so its V-slice has time) ----
            out_v = out[:, :, :, :]  # [img, rco, y, x]
            for r in (1, 2, 3, 0):
                for img in range(IMGS_PER_CORE):
                    for t in range(N_TILES):
                        acc = ps.tile(
                            [COUT, ROWS_PER_TILE * W],
                            mybir.dt.float32,
                            name="acc",
                            tag="conv",
                            bufs=4,
                        )
                        y0 = t * ROWS_PER_TILE
                        for p in range(TAPS):
                            dy, dx = p // KS, p % KS
                            rhs = xpad_v[
                                :, img, y0 + dy : y0 + dy + ROWS_PER_TILE,
                                dx : dx + W,
                            ]
                            nc.tensor.matmul(
                                acc,
                                vbuf_v[:, r, p, :],
                                rhs,
                                start=(p == 0),
                                stop=(p == TAPS - 1),
                            )
                        ot = outp.tile([COUT, ROWS_PER_TILE * W], F32, name="ot")
                        nc.vector.tensor_copy(ot, acc)
                        nc.sync.dma_start(
                            out=out_v[
                                img,
                                r * COUT : (r + 1) * COUT,
                                y0 : y0 + ROWS_PER_TILE,
                                :,
                            ].rearrange("o h w -> o (h w)"),
                            in_=ot,
                        )

    nc.finalize()
    return nc


_NC_CACHE = None


def _get_nc():
    global _NC_CACHE
    if _NC_CACHE is None:
        _NC_CACHE = _build()
    return _NC_CACHE


def kernel(
    input: np.ndarray,
    weight: np.ndarray,
    _trace: bool = False,
    _trace_cores=None,
    _result_holder: dict | None = None,
) -> np.ndarray:
    input = np.ascontiguousarray(np.asarray(input, dtype=np.float32))
    weight = np.ascontiguousarray(np.asarray(weight, dtype=np.float32))
    assert input.shape == (B, CIN, H, W), input.shape
    assert weight.shape == (COUT, CIN, KS, KS), weight.shape

    mcat = _mcat()
    ident = np.eye(CIN, dtype=np.float32)

    nc = _get_nc()
    in_maps = []
    for core in range(N_CORES):
        in_maps.append(
            {
                "x": input[core * IMGS_PER_CORE : (core + 1) * IMGS_PER_CORE],
                "w": weight,
                "mcat": mcat,
                "ident": ident,
            }
        )

    kwargs = {}
    if _trace:
        kwargs["trace"] = True
        if _trace_cores is not None:
            kwargs["trace_cores"] = _trace_cores

    res = run_bass_kernel_spmd(nc, in_maps, list(range(N_CORES)), **kwargs)
    if _result_holder is not None:
        _result_holder["res"] = res

    out = np.concatenate([res.results[c]["out"] for c in range(N_CORES)], axis=0)
    return out


# revision 7
# speedup vs baseline: 1.1244x; 1.0143x over previous
"""GCNN layer (sinc-rotated filter bank + 7x7 conv) as a Bass/Tile kernel
on 8 Trainium2 NeuronCores.

Strategy: data-parallel over batch (16 images -> 2 per core). Each core:
  1. loads the full weight (128,128,7,7), rounds it to f32r once, and
     generates the 3 rotated filter banks on-device: per output channel co,
     PE-transpose W[co] ([ci,49] -> [49,ci]) then one f32r matmul with
     Mcat = [M | M^2 | M^3] (49x147, host-precomputed sinc-rotation matrix,
     f32r-rounded on device). f32r products of pre-rounded operands are
     exact (11+11 mantissa bits accumulate in fp32), so the generated
     filters carry only the input-rounding error. Rotation 0 is the
     original weight: a bulk reshuffle copy, hidden under the conv by
     convolving rotation 0 last.
  2. zero-pads its 2 images into SBUF ([ci, img, 70, 70], f32r) and runs the
     conv as 49 shifted f32r matmuls per PSUM tile: out[co, 8rows x 64cols]
     accumulated over taps, 4 rotations x 2 images x 8 row-blocks = 64 tiles.
f32r streams the 128x512 moving operand at 1 cycle/row (4x faster than fp32)
at ~11-bit input mantissa, giving ~1.5e-4 relative output error.
"""

import numpy as np

import concourse.bacc as bacc
import concourse.mybir as mybir
from concourse.tile import TileContext
from concourse.bass_utils import run_bass_kernel_spmd

F32 = mybir.dt.float32
F32R = mybir.dt.float32r

B, CIN, H, W = 16, 128, 64, 64
COUT, KS = 128, 7
GROUP = 4
NROT = GROUP - 1  # generated rotations (r=1..3)
MCAT_N = NROT * 49 + 1  # padded to an even free dim (PSUM 8B granularity)
TAPS = KS * KS  # 49
N_CORES = 8
IMGS_PER_CORE = B // N_CORES  # 2
PAD = 3
HP = H + 2 * PAD  # 70
WP = W + 2 * PAD  # 70
ROWS_PER_TILE = 8  # 8 rows x 64 cols = 512 = one PSUM bank of fp32
N_TILES = H // ROWS_PER_TILE  # 8
COS_PER_GRP = 3  # gen psum batch: 3 * 147 * 4B = 1764B <= one 2KB bank


def _mcat() -> np.ndarray:
    """[49, 3*49] = [M | M^2 | M^3], M the sinc-interp rotation matrix.

    Matches reference._sinc_int: new_x[..., p] = sum_ab x[..., a, b] *
    sinc(tx[p]-a) * sinc(ty[p]-b), i.e. right-multiplication by
    M[(a,b), p] = sx[a,p] * sy[b,p].
    """
    k = KS
    L = k * k
    th = np.float32(90.0)  # radians, faithful to the torch module
    c, s = np.float64(np.cos(th, dtype=np.float32)), np.float64(
        np.sin(th, dtype=np.float32)
    )
    A = np.array([[c, -s], [s, c]], dtype=np.float64)
    cx = np.arange(k, dtype=np.float64) - k // 2
    grid = np.stack(np.meshgrid(cx, cx, indexing="ij"), axis=-1).reshape(L, 2).T
    t = A @ grid
    tx = t[0] + k // 2 - 1
    ty = t[1] + k // 2
    old = np.arange(k, dtype=np.float64)
    sx = np.sinc(tx[None, :] - old[:, None])  # (k, L)
    sy = np.sinc(ty[None, :] - old[:, None])
    M = (sx[:, None, :] * sy[None, :, :]).reshape(L, L)
    blocks = [M, M @ M, M @ M @ M, np.zeros((L, 1))]  # pad to even N
    return np.concatenate(blocks, axis=1).astype(np.float32)  # (49, 148)


def _build():
    nc = bacc.Bacc("TRN2")
    x_in = nc.declare_dram_parameter(
        "x", [IMGS_PER_CORE, CIN, H, W], F32, isOutput=False
    )
    w_in = nc.declare_dram_parameter("w", [COUT, CIN, KS, KS], F32, isOutput=False)
    mcat_in = nc.declare_dram_parameter(
        "mcat", [TAPS, MCAT_N], F32, isOutput=False
    )
    ident_in = nc.declare_dram_parameter("ident", [CIN, CIN], F32, isOutput=False)
    out = nc.declare_dram_parameter(
        "out", [IMGS_PER_CORE, GROUP * COUT, H, W], F32, isOutput=True
    )

    with TileContext(nc) as tc:
        with (
            tc.tile_pool(name="big", bufs=1) as big,
            tc.tile_pool(name="stage", bufs=2) as stage,
            tc.tile_pool(name="small", bufs=1) as small,
            tc.tile_pool(name="genl", bufs=3) as genl,
            tc.tile_pool(name="outp", bufs=4) as outp,
            tc.tile_pool(name="ps", bufs=1, space="PSUM") as ps,
        ):
            # ---- static SBUF buffers ----
            # f32r-rounded weights in natural layout: [ci, (co, q)]
            wr32 = big.tile([CIN, COUT * TAPS], F32R)
            wr32_v = wr32.rearrange("c (o q) -> c o q", o=COUT)
            # generated filters, conv-ready: [ci, (r, p, co)]
            vbuf = big.tile([CIN, GROUP * TAPS * COUT], F32R)
            vbuf_v = vbuf.rearrange(
                "c (r p o) -> c r p o", r=GROUP, p=TAPS, o=COUT
            )
            # padded images: [ci, (img, row, col)]
            xpad = big.tile([CIN, IMGS_PER_CORE * HP * WP], F32R)
            xpad_v = xpad.rearrange(
                "c (i h w) -> c i h w", i=IMGS_PER_CORE, h=HP, w=WP
            )
            mcat = small.tile([TAPS, MCAT_N], F32)
            mcat_r = small.tile([TAPS, MCAT_N], F32R)
            ident = small.tile([CIN, CIN], F32)
            ident_r = small.tile([CIN, CIN], F32R)

            # ---- loads ----
            nc.sync.dma_start(out=mcat, in_=mcat_in[:, :])
            nc.sync.dma_start(out=ident, in_=ident_in[:, :])
            nc.vector.tensor_copy(mcat_r, mcat)
            nc.vector.tensor_copy(ident_r, ident)

            # w[co, ci, q] -> wr32[ci, co*49+q] in 4 chunks of 32 cos so the
            # transposes can start as soon as the first chunk lands
            W_CHUNK = 32
            for c0 in range(0, COUT, W_CHUNK):
                wst = stage.tile([CIN, W_CHUNK * TAPS], F32, name="wstage")
                nc.sync.dma_start(
                    out=wst.rearrange("c (o q) -> c o q", o=W_CHUNK),
                    in_=w_in[c0 : c0 + W_CHUNK, :, :, :].rearrange(
                        "o c kh kw -> c o (kh kw)"
                    ),
                )
                nc.vector.tensor_copy(
                    wr32[:, c0 * TAPS : (c0 + W_CHUNK) * TAPS], wst
                )

            # ---- filter generation (rotations 1..3), 3 cos per psum bank ----
            co_groups = [
                (c0, min(COS_PER_GRP, COUT - c0))
                for c0 in range(0, COUT, COS_PER_GRP)
            ]
            for c0, ng in co_groups:
                trp = ps.tile(
                    [TAPS, COS_PER_GRP * CIN],
                    F32R,
                    name="trp",
                    tag="tr",
                    bufs=3,
                )
                for j in range(ng):
                    nc.tensor.transpose(
                        trp[:, j * CIN : (j + 1) * CIN],
                        wr32_v[:, c0 + j, :],
                        ident_r,
                    )
                wcoT = genl.tile([TAPS, COS_PER_GRP * CIN], F32R, name="wcoT")
                nc.vector.tensor_copy(
                    wcoT[:, : ng * CIN], trp[:, : ng * CIN]
                )
                gps = ps.tile(
                    [CIN, COS_PER_GRP * MCAT_N],
                    mybir.dt.float32,
                    name="gps",
                    tag="gen",
                    bufs=3,
                )
                for j in range(ng):
                    nc.tensor.matmul(
                        gps[:, j * MCAT_N : (j + 1) * MCAT_N],
                        wcoT[:, j * CIN : (j + 1) * CIN],
                        mcat_r,
                        start=True,
                        stop=True,
                    )
                # evac (rp-outer, co-inner): contiguous-ish writes into vbuf
                src = gps.rearrange(
                    "c (j rp) -> c j rp", j=COS_PER_GRP
                )[:, :, : NROT * TAPS].transpose([0, 2, 1])[:, :, :ng]
                dst = (
                    vbuf_v[:, 1:, :, c0 : c0 + ng]
                    .rearrange("c r p o -> c (r p) o")
                )
                nc.vector.tensor_copy(dst, src)

            # ---- input load (emitted after gen so weight DMAs win priority;
            # casts run on the otherwise-idle scalar engine) ----
            zrow = small.tile([CIN, 3 * WP], F32)
            nc.vector.memset(zrow, 0.0)
            for img in range(IMGS_PER_CORE):
                nc.scalar.copy(
                    xpad_v[:, img, 0:PAD, :],
                    zrow.rearrange("c (h w) -> c h w", h=PAD),
                )
                nc.scalar.copy(
                    xpad_v[:, img, HP - PAD : HP, :],
                    zrow.rearrange("c (h w) -> c h w", h=PAD),
                )
                nc.scalar.copy(
                    xpad_v[:, img, PAD : PAD + H, 0:PAD],
                    zrow[:, : H * PAD].rearrange("c (h w) -> c h w", h=H),
                )
                nc.scalar.copy(
                    xpad_v[:, img, PAD : PAD + H, WP - PAD : WP],
                    zrow[:, : H * PAD].rearrange("c (h w) -> c h w", h=H),
                )
            CHUNK_ROWS = 16
            for img in range(IMGS_PER_CORE):
                for c0 in range(0, H, CHUNK_ROWS):
                    st = stage.tile([CIN, CHUNK_ROWS * W], F32, name="xstage")
                    nc.sync.dma_start(
                        out=st,
                        in_=x_in[img, :, c0 : c0 + CHUNK_ROWS, :].rearrange(
                            "c h w -> c (h w)"
                        ),
                    )
                    nc.scalar.copy(
                        xpad_v[:, img, PAD + c0 : PAD + c0 + CHUNK_ROWS, PAD : PAD + W],
                        st.rearrange("c (h w) -> c h w", h=CHUNK_ROWS),
                    )

            # ---- convolution (rotation 0 last so its V-slice has time) ----
            out_v = out[:, :, :, :]  # [img, rco, y, x]
            for r in (1, 2, 3, 0):
                if r == 0:
                    # rotation 0 = the (rounded) original weight: pure
                    # reshuffle [ci,(co,q)] -> [ci,(p,co)], emitted here so
                    # the DVE runs it under the r=1..3 conv stream
                    nc.vector.tensor_copy(
                        vbuf_v[:, 0, :, :],
                        wr32_v.transpose([0, 2, 1]),
                    )
                for img in range(IMGS_PER_CORE):
                    for t in range(N_TILES):
                        acc = ps.tile(
                            [COUT, ROWS_PER_TILE * W],
                            mybir.dt.float32,
                            name="acc",
                            tag="conv",
                            bufs=2,
                        )
                        y0 = t * ROWS_PER_TILE
                        for p in range(TAPS):
                            dy, dx = p // KS, p % KS
                            rhs = xpad_v[
                                :, img, y0 + dy : y0 + dy + ROWS_PER_TILE,
                                dx : dx + W,
                            ]
                            nc.tensor.matmul(
                                acc,
                                vbuf_v[:, r, p, :],
                                rhs,
                                start=(p == 0),
                                stop=(p == TAPS - 1),
                            )
                        ot = outp.tile([COUT, ROWS_PER_TILE * W], F32, name="ot")
                        nc.vector.tensor_copy(ot, acc)
                        nc.sync.dma_start(
                            out=out_v[
                                img,
                                r * COUT : (r + 1) * COUT,
                                y0 : y0 + ROWS_PER_TILE,
                                :,
                            ].rearrange("o h w -> o (h w)"),
                            in_=ot,
                        )

    nc.finalize()
    return nc


_NC_CACHE = None


def _get_nc():
    global _NC_CACHE
    if _NC_CACHE is None:
        _NC_CACHE = _build()
    return _NC_CACHE


def kernel(
    input: np.ndarray,
    weight: np.ndarray,
    _trace: bool = False,
    _trace_cores=None,
    _result_holder: dict | None = None,
) -> np.ndarray:
    input = np.ascontiguousarray(np.asarray(input, dtype=np.float32))
    weight = np.ascontiguousarray(np.asarray(weight, dtype=np.float32))
    assert input.shape == (B, CIN, H, W), input.shape
    assert weight.shape == (COUT, CIN, KS, KS), weight.shape

    mcat = _mcat()
    ident = np.eye(CIN, dtype=np.float32)

    nc = _get_nc()
    in_maps = []
    for core in range(N_CORES):
        in_maps.append(
            {
                "x": input[core * IMGS_PER_CORE : (core + 1) * IMGS_PER_CORE],
                "w": weight,
                "mcat": mcat,
                "ident": ident,
            }
        )

    kwargs = {}
    if _trace:
        kwargs["trace"] = True
        if _trace_cores is not None:
            kwargs["trace_cores"] = _trace_cores

    res = run_bass_kernel_spmd(nc, in_maps, list(range(N_CORES)), **kwargs)
    if _result_holder is not None:
        _result_holder["res"] = res

    out = np.concatenate([res.results[c]["out"] for c in range(N_CORES)], axis=0)
    return out


# revision 8
# speedup vs baseline: 1.1257x; 1.0012x over previous
"""GCNN layer (sinc-rotated filter bank + 7x7 conv) as a Bass/Tile kernel
on 8 Trainium2 NeuronCores.

Strategy: data-parallel over batch (16 images -> 2 per core). Each core:
  1. loads the full weight (128,128,7,7), rounds it to f32r once, and
     generates the 3 rotated filter banks on-device: per output channel co,
     PE-transpose W[co] ([ci,49] -> [49,ci]) then one f32r matmul with
     Mcat = [M | M^2 | M^3] (49x147, host-precomputed sinc-rotation matrix,
     f32r-rounded on device). f32r products of pre-rounded operands are
     exact (11+11 mantissa bits accumulate in fp32), so the generated
     filters carry only the input-rounding error. Rotation 0 is the
     original weight: a bulk reshuffle copy, hidden under the conv by
     convolving rotation 0 last.
  2. zero-pads its 2 images into SBUF ([ci, img, 70, 70], f32r) and runs the
     conv as 49 shifted f32r matmuls per PSUM tile: out[co, 8rows x 64cols]
     accumulated over taps, 4 rotations x 2 images x 8 row-blocks = 64 tiles.
f32r streams the 128x512 moving operand at 1 cycle/row (4x faster than fp32)
at ~11-bit input mantissa, giving ~1.5e-4 relative output error.
"""

import numpy as np

import concourse.bacc as bacc
import concourse.mybir as mybir
from concourse.tile import TileContext
from concourse.bass_utils import run_bass_kernel_spmd

F32 = mybir.dt.float32
F32R = mybir.dt.float32r

B, CIN, H, W = 16, 128, 64, 64
COUT, KS = 128, 7
GROUP = 4
NROT = GROUP - 1  # generated rotations (r=1..3)
MCAT_N = NROT * 49 + 1  # padded to an even free dim (PSUM 8B granularity)
TAPS = KS * KS  # 49
N_CORES = 8
IMGS_PER_CORE = B // N_CORES  # 2
PAD = 3
HP = H + 2 * PAD  # 70
WP = W + 2 * PAD  # 70
ROWS_PER_TILE = 8  # 8 rows x 64 cols = 512 = one PSUM bank of fp32
N_TILES = H // ROWS_PER_TILE  # 8
COS_PER_GRP = 3  # gen psum batch: 3 * 147 * 4B = 1764B <= one 2KB bank


def _mcat() -> np.ndarray:
    """[49, 3*49] = [M | M^2 | M^3], M the sinc-interp rotation matrix.

    Matches reference._sinc_int: new_x[..., p] = sum_ab x[..., a, b] *
    sinc(tx[p]-a) * sinc(ty[p]-b), i.e. right-multiplication by
    M[(a,b), p] = sx[a,p] * sy[b,p].
    """
    k = KS
    L = k * k
    th = np.float32(90.0)  # radians, faithful to the torch module
    c, s = np.float64(np.cos(th, dtype=np.float32)), np.float64(
        np.sin(th, dtype=np.float32)
    )
    A = np.array([[c, -s], [s, c]], dtype=np.float64)
    cx = np.arange(k, dtype=np.float64) - k // 2
    grid = np.stack(np.meshgrid(cx, cx, indexing="ij"), axis=-1).reshape(L, 2).T
    t = A @ grid
    tx = t[0] + k // 2 - 1
    ty = t[1] + k // 2
    old = np.arange(k, dtype=np.float64)
    sx = np.sinc(tx[None, :] - old[:, None])  # (k, L)
    sy = np.sinc(ty[None, :] - old[:, None])
    M = (sx[:, None, :] * sy[None, :, :]).reshape(L, L)
    blocks = [M, M @ M, M @ M @ M, np.zeros((L, 1))]  # pad to even N
    return np.concatenate(blocks, axis=1).astype(np.float32)  # (49, 148)


def _build():
    nc = bacc.Bacc("TRN2")
    x_in = nc.declare_dram_parameter(
        "x", [IMGS_PER_CORE, CIN, H, W], F32, isOutput=False
    )
    w_in = nc.declare_dram_parameter("w", [COUT, CIN, KS, KS], F32, isOutput=False)
    mcat_in = nc.declare_dram_parameter(
        "mcat", [TAPS, MCAT_N], F32, isOutput=False
    )
    ident_in = nc.declare_dram_parameter("ident", [CIN, CIN], F32, isOutput=False)
    out = nc.declare_dram_parameter(
        "out", [IMGS_PER_CORE, GROUP * COUT, H, W], F32, isOutput=True
    )

    with TileContext(nc) as tc:
        with (
            tc.tile_pool(name="big", bufs=1) as big,
            tc.tile_pool(name="stage", bufs=2) as stage,
            tc.tile_pool(name="small", bufs=1) as small,
            tc.tile_pool(name="genl", bufs=3) as genl,
            tc.tile_pool(name="outp", bufs=4) as outp,
            tc.tile_pool(name="ps", bufs=1, space="PSUM") as ps,
        ):
            # ---- static SBUF buffers ----
            # weights in natural layout: [ci, (co, q)] (f32; rounding to f32r
            # happens on the PSUM evacuations)
            wnat = big.tile([CIN, COUT * TAPS], F32)
            wnat_v = wnat.rearrange("c (o q) -> c o q", o=COUT)
            # generated filters, conv-ready: [ci, (r, p, co)]
            vbuf = big.tile([CIN, GROUP * TAPS * COUT], F32R)
            vbuf_v = vbuf.rearrange(
                "c (r p o) -> c r p o", r=GROUP, p=TAPS, o=COUT
            )
            # padded images: [ci, (img, row, col)]
            xpad = big.tile([CIN, IMGS_PER_CORE * HP * WP], F32R)
            xpad_v = xpad.rearrange(
                "c (i h w) -> c i h w", i=IMGS_PER_CORE, h=HP, w=WP
            )
            mcat = small.tile([TAPS, MCAT_N], F32)
            mcat_r = small.tile([TAPS, MCAT_N], F32R)
            ident = small.tile([CIN, CIN], F32)

            # ---- loads ----
            nc.sync.dma_start(out=mcat, in_=mcat_in[:, :])
            nc.sync.dma_start(out=ident, in_=ident_in[:, :])
            nc.vector.tensor_copy(mcat_r, mcat)

            # w[co, ci, q] -> wnat[ci, co*49+q], geometric chunks so the
            # first transposes can start ~2us after launch
            c0 = 0
            for w_chunk in (8, 8, 16, 32, 64):
                nc.sync.dma_start(
                    out=wnat_v[:, c0 : c0 + w_chunk, :],
                    in_=w_in[c0 : c0 + w_chunk, :, :, :].rearrange(
                        "o c kh kw -> c o (kh kw)"
                    ),
                )
                c0 += w_chunk

            # ---- filter generation (rotations 1..3), 3 cos per psum bank ----
            co_groups = [
                (c0, min(COS_PER_GRP, COUT - c0))
                for c0 in range(0, COUT, COS_PER_GRP)
            ]
            for c0, ng in co_groups:
                trp = ps.tile(
                    [TAPS, COS_PER_GRP * CIN],
                    mybir.dt.float32,
                    name="trp",
                    tag="tr",
                    bufs=3,
                )
                for j in range(ng):
                    nc.tensor.transpose(
                        trp[:, j * CIN : (j + 1) * CIN],
                        wnat_v[:, c0 + j, :],
                        ident,
                    )
                # ACT evac: casts (rounds) to f32r off the DVE critical path
                wcoT = genl.tile([TAPS, COS_PER_GRP * CIN], F32R, name="wcoT")
                nc.scalar.copy(wcoT[:, : ng * CIN], trp[:, : ng * CIN])
                gps = ps.tile(
                    [CIN, COS_PER_GRP * MCAT_N],
                    mybir.dt.float32,
                    name="gps",
                    tag="gen",
                    bufs=3,
                )
                for j in range(ng):
                    nc.tensor.matmul(
                        gps[:, j * MCAT_N : (j + 1) * MCAT_N],
                        wcoT[:, j * CIN : (j + 1) * CIN],
                        mcat_r,
                        start=True,
                        stop=True,
                    )
                # evac (rp-outer, co-inner): contiguous-ish writes into vbuf
                src = gps.rearrange(
                    "c (j rp) -> c j rp", j=COS_PER_GRP
                )[:, :, : NROT * TAPS].transpose([0, 2, 1])[:, :, :ng]
                dst = (
                    vbuf_v[:, 1:, :, c0 : c0 + ng]
                    .rearrange("c r p o -> c (r p) o")
                )
                nc.vector.tensor_copy(dst, src)

            # ---- input load (emitted after gen so weight DMAs win priority;
            # casts run on the otherwise-idle scalar engine) ----
            zrow = small.tile([CIN, 3 * WP], F32)
            nc.vector.memset(zrow, 0.0)
            for img in range(IMGS_PER_CORE):
                nc.scalar.copy(
                    xpad_v[:, img, 0:PAD, :],
                    zrow.rearrange("c (h w) -> c h w", h=PAD),
                )
                nc.scalar.copy(
                    xpad_v[:, img, HP - PAD : HP, :],
                    zrow.rearrange("c (h w) -> c h w", h=PAD),
                )
                nc.scalar.copy(
                    xpad_v[:, img, PAD : PAD + H, 0:PAD],
                    zrow[:, : H * PAD].rearrange("c (h w) -> c h w", h=H),
                )
                nc.scalar.copy(
                    xpad_v[:, img, PAD : PAD + H, WP - PAD : WP],
                    zrow[:, : H * PAD].rearrange("c (h w) -> c h w", h=H),
                )
            CHUNK_ROWS = 16
            for img in range(IMGS_PER_CORE):
                for c0 in range(0, H, CHUNK_ROWS):
                    st = stage.tile([CIN, CHUNK_ROWS * W], F32, name="xstage")
                    nc.sync.dma_start(
                        out=st,
                        in_=x_in[img, :, c0 : c0 + CHUNK_ROWS, :].rearrange(
                            "c h w -> c (h w)"
                        ),
                    )
                    nc.scalar.copy(
                        xpad_v[:, img, PAD + c0 : PAD + c0 + CHUNK_ROWS, PAD : PAD + W],
                        st.rearrange("c (h w) -> c h w", h=CHUNK_ROWS),
                    )

            # ---- convolution (rotation 0 last so its V-slice has time) ----
            out_v = out[:, :, :, :]  # [img, rco, y, x]
            for r in (1, 2, 3, 0):
                if r == 0:
                    # rotation 0 = the (rounded) original weight: pure
                    # reshuffle [ci,(co,q)] -> [ci,(p,co)], emitted here so
                    # the DVE runs it under the r=1..3 conv stream
                    nc.vector.tensor_copy(
                        vbuf_v[:, 0, :, :],
                        wnat_v.transpose([0, 2, 1]),
                    )
                for img in range(IMGS_PER_CORE):
                    for t in range(N_TILES):
                        acc = ps.tile(
                            [COUT, ROWS_PER_TILE * W],
                            mybir.dt.float32,
                            name="acc",
                            tag="conv",
                            bufs=2,
                        )
                        y0 = t * ROWS_PER_TILE
                        for p in range(TAPS):
                            dy, dx = p // KS, p % KS
                            rhs = xpad_v[
                                :, img, y0 + dy : y0 + dy + ROWS_PER_TILE,
                                dx : dx + W,
                            ]
                            nc.tensor.matmul(
                                acc,
                                vbuf_v[:, r, p, :],
                                rhs,
                                start=(p == 0),
                                stop=(p == TAPS - 1),
                            )
                        ot = outp.tile([COUT, ROWS_PER_TILE * W], F32, name="ot")
                        nc.vector.tensor_copy(ot, acc)
                        nc.sync.dma_start(
                            out=out_v[
                                img,
                                r * COUT : (r + 1) * COUT,
                                y0 : y0 + ROWS_PER_TILE,
                                :,
                            ].rearrange("o h w -> o (h w)"),
                            in_=ot,
                        )

    nc.finalize()
    return nc


_NC_CACHE = None


def _get_nc():
    global _NC_CACHE
    if _NC_CACHE is None:
        _NC_CACHE = _build()
    return _NC_CACHE


def kernel(
    input: np.ndarray,
    weight: np.ndarray,
    _trace: bool = False,
    _trace_cores=None,
    _result_holder: dict | None = None,
) -> np.ndarray:
    input = np.ascontiguousarray(np.asarray(input, dtype=np.float32))
    weight = np.ascontiguousarray(np.asarray(weight, dtype=np.float32))
    assert input.shape == (B, CIN, H, W), input.shape
    assert weight.shape == (COUT, CIN, KS, KS), weight.shape

    mcat = _mcat()
    ident = np.eye(CIN, dtype=np.float32)

    nc = _get_nc()
    in_maps = []
    for core in range(N_CORES):
        in_maps.append(
            {
                "x": input[core * IMGS_PER_CORE : (core + 1) * IMGS_PER_CORE],
                "w": weight,
                "mcat": mcat,
                "ident": ident,
            }
        )

    kwargs = {}
    if _trace:
        kwargs["trace"] = True
        if _trace_cores is not None:
            kwargs["trace_cores"] = _trace_cores

    res = run_bass_kernel_spmd(nc, in_maps, list(range(N_CORES)), **kwargs)
    if _result_holder is not None:
        _result_holder["res"] = res

    out = np.concatenate([res.results[c]["out"] for c in range(N_CORES)], axis=0)
    return out


# revision 9
# speedup vs baseline: 1.1263x; 1.0006x over previous
"""GCNN layer (sinc-rotated filter bank + 7x7 conv) as a Bass/Tile kernel
on 8 Trainium2 NeuronCores.

Strategy: data-parallel over batch (16 images -> 2 per core). Each core:
  1. loads the full weight (128,128,7,7), rounds it to f32r once, and
     generates the 3 rotated filter banks on-device: per output channel co,
     PE-transpose W[co] ([ci,49] -> [49,ci]) then one f32r matmul with
     Mcat = [M | M^2 | M^3] (49x147, host-precomputed sinc-rotation matrix,
     f32r-rounded on device). f32r products of pre-rounded operands are
     exact (11+11 mantissa bits accumulate in fp32), so the generated
     filters carry only the input-rounding error. Rotation 0 is the
     original weight: a bulk reshuffle copy, hidden under the conv by
     convolving rotation 0 last.
  2. zero-pads its 2 images into SBUF ([ci, img, 70, 70], f32r) and runs the
     conv as 49 shifted f32r matmuls per PSUM tile: out[co, 8rows x 64cols]
     accumulated over taps, 4 rotations x 2 images x 8 row-blocks = 64 tiles.
f32r streams the 128x512 moving operand at 1 cycle/row (4x faster than fp32)
at ~11-bit input mantissa, giving ~1.5e-4 relative output error.
"""

import numpy as np

import concourse.bacc as bacc
import concourse.mybir as mybir
from concourse.tile import TileContext
from concourse.bass_utils import run_bass_kernel_spmd

F32 = mybir.dt.float32
F32R = mybir.dt.float32r

B, CIN, H, W = 16, 128, 64, 64
COUT, KS = 128, 7
GROUP = 4
NROT = GROUP - 1  # generated rotations (r=1..3)
MCAT_N = NROT * 49 + 1  # padded to an even free dim (PSUM 8B granularity)
TAPS = KS * KS  # 49
N_CORES = 8
IMGS_PER_CORE = B // N_CORES  # 2
PAD = 3
HP = H + 2 * PAD  # 70
WP = W + 2 * PAD  # 70
ROWS_PER_TILE = 8  # 8 rows x 64 cols = 512 = one PSUM bank of fp32
N_TILES = H // ROWS_PER_TILE  # 8
COS_PER_GRP = 3  # gen psum batch: 3 * 147 * 4B = 1764B <= one 2KB bank


def _mcat() -> np.ndarray:
    """[49, 3*49] = [M | M^2 | M^3], M the sinc-interp rotation matrix.

    Matches reference._sinc_int: new_x[..., p] = sum_ab x[..., a, b] *
    sinc(tx[p]-a) * sinc(ty[p]-b), i.e. right-multiplication by
    M[(a,b), p] = sx[a,p] * sy[b,p].
    """
    k = KS
    L = k * k
    th = np.float32(90.0)  # radians, faithful to the torch module
    c, s = np.float64(np.cos(th, dtype=np.float32)), np.float64(
        np.sin(th, dtype=np.float32)
    )
    A = np.array([[c, -s], [s, c]], dtype=np.float64)
    cx = np.arange(k, dtype=np.float64) - k // 2
    grid = np.stack(np.meshgrid(cx, cx, indexing="ij"), axis=-1).reshape(L, 2).T
    t = A @ grid
    tx = t[0] + k // 2 - 1
    ty = t[1] + k // 2
    old = np.arange(k, dtype=np.float64)
    sx = np.sinc(tx[None, :] - old[:, None])  # (k, L)
    sy = np.sinc(ty[None, :] - old[:, None])
    M = (sx[:, None, :] * sy[None, :, :]).reshape(L, L)
    blocks = [M, M @ M, M @ M @ M, np.zeros((L, 1))]  # pad to even N
    return np.concatenate(blocks, axis=1).astype(np.float32)  # (49, 148)


def _build():
    nc = bacc.Bacc("TRN2")
    x_in = nc.declare_dram_parameter(
        "x", [IMGS_PER_CORE, CIN, H, W], F32, isOutput=False
    )
    w_in = nc.declare_dram_parameter("w", [COUT, CIN, KS, KS], F32, isOutput=False)
    mcat_in = nc.declare_dram_parameter(
        "mcat", [TAPS, MCAT_N], F32, isOutput=False
    )
    ident_in = nc.declare_dram_parameter("ident", [CIN, CIN], F32, isOutput=False)
    out = nc.declare_dram_parameter(
        "out", [IMGS_PER_CORE, GROUP * COUT, H, W], F32, isOutput=True
    )

    with TileContext(nc) as tc:
        with (
            tc.tile_pool(name="big", bufs=1) as big,
            tc.tile_pool(name="stage", bufs=2) as stage,
            tc.tile_pool(name="small", bufs=1) as small,
            tc.tile_pool(name="genl", bufs=3) as genl,
            tc.tile_pool(name="outp", bufs=4) as outp,
            tc.tile_pool(name="ps", bufs=1, space="PSUM") as ps,
        ):
            # ---- static SBUF buffers ----
            # weights in natural layout: [ci, (co, q)] (f32; rounding to f32r
            # happens on the PSUM evacuations)
            wnat = big.tile([CIN, COUT * TAPS], F32)
            wnat_v = wnat.rearrange("c (o q) -> c o q", o=COUT)
            # generated filters, conv-ready: [ci, (r, p, co)]
            vbuf = big.tile([CIN, GROUP * TAPS * COUT], F32R)
            vbuf_v = vbuf.rearrange(
                "c (r p o) -> c r p o", r=GROUP, p=TAPS, o=COUT
            )
            # padded images: [ci, (img, row, col)]
            xpad = big.tile([CIN, IMGS_PER_CORE * HP * WP], F32R)
            xpad_v = xpad.rearrange(
                "c (i h w) -> c i h w", i=IMGS_PER_CORE, h=HP, w=WP
            )
            mcat = small.tile([TAPS, MCAT_N], F32)
            mcat_r = small.tile([TAPS, MCAT_N], F32R)
            ident = small.tile([CIN, CIN], F32)

            # ---- loads ----
            nc.sync.dma_start(out=mcat, in_=mcat_in[:, :])
            nc.sync.dma_start(out=ident, in_=ident_in[:, :])
            nc.vector.tensor_copy(mcat_r, mcat)

            # w[co, ci, q] -> wnat[ci, co*49+q], geometric chunks so the
            # first transposes can start ~2us after launch
            c0 = 0
            for w_chunk in (4, 4, 8, 16, 16, 16, 16, 16, 16, 16):
                nc.sync.dma_start(
                    out=wnat_v[:, c0 : c0 + w_chunk, :],
                    in_=w_in[c0 : c0 + w_chunk, :, :, :].rearrange(
                        "o c kh kw -> c o (kh kw)"
                    ),
                )
                c0 += w_chunk

            # ---- filter generation (rotations 1..3), 3 cos per psum bank ----
            co_groups = [
                (c0, min(COS_PER_GRP, COUT - c0))
                for c0 in range(0, COUT, COS_PER_GRP)
            ]
            for c0, ng in co_groups:
                trp = ps.tile(
                    [TAPS, COS_PER_GRP * CIN],
                    mybir.dt.float32,
                    name="trp",
                    tag="tr",
                    bufs=3,
                )
                for j in range(ng):
                    nc.tensor.transpose(
                        trp[:, j * CIN : (j + 1) * CIN],
                        wnat_v[:, c0 + j, :],
                        ident,
                    )
                # ACT evac: casts (rounds) to f32r off the DVE critical path
                wcoT = genl.tile([TAPS, COS_PER_GRP * CIN], F32R, name="wcoT")
                nc.scalar.copy(wcoT[:, : ng * CIN], trp[:, : ng * CIN])
                gps = ps.tile(
                    [CIN, COS_PER_GRP * MCAT_N],
                    mybir.dt.float32,
                    name="gps",
                    tag="gen",
                    bufs=3,
                )
                for j in range(ng):
                    nc.tensor.matmul(
                        gps[:, j * MCAT_N : (j + 1) * MCAT_N],
                        wcoT[:, j * CIN : (j + 1) * CIN],
                        mcat_r,
                        start=True,
                        stop=True,
                    )
                # evac (rp-outer, co-inner): contiguous-ish writes into vbuf
                src = gps.rearrange(
                    "c (j rp) -> c j rp", j=COS_PER_GRP
                )[:, :, : NROT * TAPS].transpose([0, 2, 1])[:, :, :ng]
                dst = (
                    vbuf_v[:, 1:, :, c0 : c0 + ng]
                    .rearrange("c r p o -> c (r p) o")
                )
                nc.vector.tensor_copy(dst, src)

            # ---- input load (scheduled after the weight DMAs via a
            # model-time floor; casts run on the otherwise-idle scalar
            # engine) ----
            tc.tile_set_cur_wait(0.018)
            zrow = small.tile([CIN, 3 * WP], F32)
            nc.vector.memset(zrow, 0.0)
            for img in range(IMGS_PER_CORE):
                nc.scalar.copy(
                    xpad_v[:, img, 0:PAD, :],
                    zrow.rearrange("c (h w) -> c h w", h=PAD),
                )
                nc.scalar.copy(
                    xpad_v[:, img, HP - PAD : HP, :],
                    zrow.rearrange("c (h w) -> c h w", h=PAD),
                )
                nc.scalar.copy(
                    xpad_v[:, img, PAD : PAD + H, 0:PAD],
                    zrow[:, : H * PAD].rearrange("c (h w) -> c h w", h=H),
                )
                nc.scalar.copy(
                    xpad_v[:, img, PAD : PAD + H, WP - PAD : WP],
                    zrow[:, : H * PAD].rearrange("c (h w) -> c h w", h=H),
                )
            CHUNK_ROWS = 16
            for img in range(IMGS_PER_CORE):
                for c0 in range(0, H, CHUNK_ROWS):
                    st = stage.tile([CIN, CHUNK_ROWS * W], F32, name="xstage")
                    nc.sync.dma_start(
                        out=st,
                        in_=x_in[img, :, c0 : c0 + CHUNK_ROWS, :].rearrange(
                            "c h w -> c (h w)"
                        ),
                    )
                    nc.scalar.copy(
                        xpad_v[:, img, PAD + c0 : PAD + c0 + CHUNK_ROWS, PAD : PAD + W],
                        st.rearrange("c (h w) -> c h w", h=CHUNK_ROWS),
                    )

            tc.tile_set_cur_wait(0, enable=False)
            tc.cur_wait_ts = None
            # ---- convolution (rotation 0 last so its V-slice has time) ----
            out_v = out[:, :, :, :]  # [img, rco, y, x]
            for r in (1, 2, 3, 0):
                if r == 0:
                    # rotation 0 = the (rounded) original weight: pure
                    # reshuffle [ci,(co,q)] -> [ci,(p,co)], emitted here so
                    # the DVE runs it under the r=1..3 conv stream
                    nc.vector.tensor_copy(
                        vbuf_v[:, 0, :, :],
                        wnat_v.transpose([0, 2, 1]),
                    )
                for img in range(IMGS_PER_CORE):
                    for t in range(N_TILES):
                        acc = ps.tile(
                            [COUT, ROWS_PER_TILE * W],
                            mybir.dt.float32,
                            name="acc",
                            tag="conv",
                            bufs=2,
                        )
                        y0 = t * ROWS_PER_TILE
                        for p in range(TAPS):
                            dy, dx = p // KS, p % KS
                            rhs = xpad_v[
                                :, img, y0 + dy : y0 + dy + ROWS_PER_TILE,
                                dx : dx + W,
                            ]
                            nc.tensor.matmul(
                                acc,
                                vbuf_v[:, r, p, :],
                                rhs,
                                start=(p == 0),
                                stop=(p == TAPS - 1),
                            )
                        ot = outp.tile([COUT, ROWS_PER_TILE * W], F32, name="ot")
                        nc.vector.tensor_copy(ot, acc)
                        nc.sync.dma_start(
                            out=out_v[
                                img,
                                r * COUT : (r + 1) * COUT,
                                y0 : y0 + ROWS_PER_TILE,
                                :,
                            ].rearrange("o h w -> o (h w)"),
                            in_=ot,
                        )

    nc.finalize()
    return nc


_NC_CACHE = None


def _get_nc():
    global _NC_CACHE
    if _NC_CACHE is None:
        _NC_CACHE = _build()
    return _NC_CACHE


def kernel(
    input: np.ndarray,
    weight: np.ndarray,
    _trace: bool = False,
    _trace_cores=None,
    _result_holder: dict | None = None,
) -> np.ndarray:
    input = np.ascontiguousarray(np.asarray(input, dtype=np.float32))
    weight = np.ascontiguousarray(np.asarray(weight, dtype=np.float32))
    assert input.shape == (B, CIN, H, W), input.shape
    assert weight.shape == (COUT, CIN, KS, KS), weight.shape

    mcat = _mcat()
    ident = np.eye(CIN, dtype=np.float32)

    nc = _get_nc()
    in_maps = []
    for core in range(N_CORES):
        in_maps.append(
            {
                "x": input[core * IMGS_PER_CORE : (core + 1) * IMGS_PER_CORE],
                "w": weight,
                "mcat": mcat,
                "ident": ident,
            }
        )

    kwargs = {}
    if _trace:
        kwargs["trace"] = True
        if _trace_cores is not None:
            kwargs["trace_cores"] = _trace_cores

    res = run_bass_kernel_spmd(nc, in_maps, list(range(N_CORES)), **kwargs)
    if _result_holder is not None:
        _result_holder["res"] = res

    out = np.concatenate([res.results[c]["out"] for c in range(N_CORES)], axis=0)
    return out


# revision 11
# speedup vs baseline: 1.1274x; 1.0009x over previous
"""GCNN layer (sinc-rotated filter bank + 7x7 conv) as a Bass/Tile kernel
on 8 Trainium2 NeuronCores.

Strategy: data-parallel over batch (16 images -> 2 per core). Each core:
  1. loads the full weight (128,128,7,7), rounds it to f32r once, and
     generates the 3 rotated filter banks on-device: per output channel co,
     PE-transpose W[co] ([ci,49] -> [49,ci]) then one f32r matmul with
     Mcat = [M | M^2 | M^3] (49x147, host-precomputed sinc-rotation matrix,
     f32r-rounded on device). f32r products of pre-rounded operands are
     exact (11+11 mantissa bits accumulate in fp32), so the generated
     filters carry only the input-rounding error. Rotation 0 is the
     original weight: a bulk reshuffle copy, hidden under the conv by
     convolving rotation 0 last.
  2. zero-pads its 2 images into SBUF ([ci, img, 70, 70], f32r) and runs the
     conv as 49 shifted f32r matmuls per PSUM tile: out[co, 8rows x 64cols]
     accumulated over taps, 4 rotations x 2 images x 8 row-blocks = 64 tiles.
f32r streams the 128x512 moving operand at 1 cycle/row (4x faster than fp32)
at ~11-bit input mantissa, giving ~1.5e-4 relative output error.
"""

import numpy as np

import concourse.bacc as bacc
import concourse.mybir as mybir
from concourse.tile import TileContext
from concourse.bass_utils import run_bass_kernel_spmd

F32 = mybir.dt.float32
F32R = mybir.dt.float32r

B, CIN, H, W = 16, 128, 64, 64
COUT, KS = 128, 7
GROUP = 4
NROT = GROUP - 1  # generated rotations (r=1..3)
MCAT_N = NROT * 49 + 1  # padded to an even free dim (PSUM 8B granularity)
QPAD = 64  # tap dim padded to 64 so transposes batch 2 cos per PE op
TAPS = KS * KS  # 49
N_CORES = 8
IMGS_PER_CORE = B // N_CORES  # 2
PAD = 3
HP = H + 2 * PAD  # 70
WP = W + 2 * PAD  # 70
ROWS_PER_TILE = 8  # 8 rows x 64 cols = 512 = one PSUM bank of fp32
N_TILES = H // ROWS_PER_TILE  # 8
COS_PER_GRP = 3  # gen psum batch: 3 * 147 * 4B = 1764B <= one 2KB bank


def _mcat() -> np.ndarray:
    """[49, 3*49] = [M | M^2 | M^3], M the sinc-interp rotation matrix.

    Matches reference._sinc_int: new_x[..., p] = sum_ab x[..., a, b] *
    sinc(tx[p]-a) * sinc(ty[p]-b), i.e. right-multiplication by
    M[(a,b), p] = sx[a,p] * sy[b,p].
    """
    k = KS
    L = k * k
    th = np.float32(90.0)  # radians, faithful to the torch module
    c, s = np.float64(np.cos(th, dtype=np.float32)), np.float64(
        np.sin(th, dtype=np.float32)
    )
    A = np.array([[c, -s], [s, c]], dtype=np.float64)
    cx = np.arange(k, dtype=np.float64) - k // 2
    grid = np.stack(np.meshgrid(cx, cx, indexing="ij"), axis=-1).reshape(L, 2).T
    t = A @ grid
    tx = t[0] + k // 2 - 1
    ty = t[1] + k // 2
    old = np.arange(k, dtype=np.float64)
    sx = np.sinc(tx[None, :] - old[:, None])  # (k, L)
    sy = np.sinc(ty[None, :] - old[:, None])
    M = (sx[:, None, :] * sy[None, :, :]).reshape(L, L)
    blocks = [M, M @ M, M @ M @ M, np.zeros((L, 1))]  # pad to even N
    mc = np.concatenate(blocks, axis=1)  # (49, 148)
    out = np.zeros((QPAD, mc.shape[1]))  # zero rows 49..63 kill the pad taps
    out[:L] = mc
    return out.astype(np.float32)


def _build():
    nc = bacc.Bacc("TRN2")
    x_in = nc.declare_dram_parameter(
        "x", [IMGS_PER_CORE, CIN, H, W], F32, isOutput=False
    )
    w_in = nc.declare_dram_parameter("w", [COUT, CIN, KS, KS], F32, isOutput=False)
    mcat_in = nc.declare_dram_parameter(
        "mcat", [QPAD, MCAT_N], F32, isOutput=False
    )
    ident_in = nc.declare_dram_parameter("ident", [CIN, CIN], F32, isOutput=False)
    out = nc.declare_dram_parameter(
        "out", [IMGS_PER_CORE, GROUP * COUT, H, W], F32, isOutput=True
    )

    with TileContext(nc) as tc:
        with (
            tc.tile_pool(name="big", bufs=1) as big,
            tc.tile_pool(name="stage", bufs=2) as stage,
            tc.tile_pool(name="small", bufs=1) as small,
            tc.tile_pool(name="genl", bufs=3) as genl,
            tc.tile_pool(name="outp", bufs=4) as outp,
            tc.tile_pool(name="ps", bufs=1, space="PSUM") as ps,
        ):
            # ---- static SBUF buffers ----
            # weights in natural layout, tap dim padded to 64: [ci, (co, 64)]
            # (f32; rounding to f32r happens on the PSUM evacuations)
            wnat = big.tile([CIN, COUT * QPAD], F32)
            wnat_v = wnat.rearrange("c (o q) -> c o q", o=COUT)
            # generated filters, conv-ready: [ci, (r, p, co)]
            vbuf = big.tile([CIN, GROUP * TAPS * COUT], F32R)
            vbuf_v = vbuf.rearrange(
                "c (r p o) -> c r p o", r=GROUP, p=TAPS, o=COUT
            )
            # padded images: [ci, (img, row, col)]
            xpad = big.tile([CIN, IMGS_PER_CORE * HP * WP], F32R)
            xpad_v = xpad.rearrange(
                "c (i h w) -> c i h w", i=IMGS_PER_CORE, h=HP, w=WP
            )
            mcat = small.tile([2 * QPAD, MCAT_N], F32)
            mcat_r = small.tile([2 * QPAD, MCAT_N], F32R)
            ident = small.tile([CIN, CIN], F32)

            # ---- loads ----
            nc.sync.dma_start(out=mcat[0:QPAD, :], in_=mcat_in[:, :])
            nc.sync.dma_start(out=mcat[QPAD : 2 * QPAD, :], in_=mcat_in[:, :])
            nc.sync.dma_start(out=ident, in_=ident_in[:, :])
            nc.vector.tensor_copy(mcat_r, mcat)

            # w[co, ci, q] -> wnat[ci, co*49+q], geometric chunks so the
            # first transposes can start ~2us after launch
            nc.gpsimd.memset(wnat_v[:, :, TAPS:QPAD], 0.0)
            c0 = 0
            for w_chunk in (4, 4, 8, 16, 16, 16, 16, 16, 16, 16):
                nc.sync.dma_start(
                    out=wnat_v[:, c0 : c0 + w_chunk, 0:TAPS],
                    in_=w_in[c0 : c0 + w_chunk, :, :, :].rearrange(
                        "o c kh kw -> c o (kh kw)"
                    ),
                )
                c0 += w_chunk

            # ---- filter generation (rotations 1..3), 3 cos per psum bank ----
            co_groups = [
                (c0, min(COS_PER_GRP, COUT - c0))
                for c0 in range(0, COUT, COS_PER_GRP)
            ]
            for c0, ng in co_groups:
                trp = ps.tile(
                    [TAPS, COS_PER_GRP * CIN],
                    mybir.dt.float32,
                    name="trp",
                    tag="tr",
                    bufs=3,
                )
                for j in range(ng):
                    nc.tensor.transpose(
                        trp[:, j * CIN : (j + 1) * CIN],
                        wnat_v[:, c0 + j, 0:TAPS],
                        ident,
                    )
                # ACT evac: casts (rounds) to f32r off the DVE critical path
                wcoT = genl.tile([TAPS, COS_PER_GRP * CIN], F32R, name="wcoT")
                nc.scalar.copy(wcoT[:, : ng * CIN], trp[:, : ng * CIN])
                gps = ps.tile(
                    [CIN, COS_PER_GRP * MCAT_N],
                    mybir.dt.float32,
                    name="gps",
                    tag="gen",
                    bufs=3,
                )
                for j in range(ng):
                    nc.tensor.matmul(
                        gps[:, j * MCAT_N : (j + 1) * MCAT_N],
                        wcoT[:, j * CIN : (j + 1) * CIN],
                        mcat_r[0:TAPS, :],
                        start=True,
                        stop=True,
                    )
                # evac (rp-outer, co-inner): contiguous-ish writes into vbuf
                esrc = gps.rearrange("c (j rp) -> c j rp", j=COS_PER_GRP)[
                    :, :, : NROT * TAPS
                ].transpose([0, 2, 1])[:, :, :ng]
                edst = vbuf_v[:, 1:, :, c0 : c0 + ng].rearrange(
                    "c r p o -> c (r p) o"
                )
                nc.vector.tensor_copy(edst, esrc)

            # ---- input load (scheduled after the weight DMAs via a
            # model-time floor; casts run on the otherwise-idle scalar
            # engine) ----
            tc.tile_set_cur_wait(0.028)
            zrow = small.tile([CIN, 3 * WP], F32)
            nc.vector.memset(zrow, 0.0)
            for img in range(IMGS_PER_CORE):
                nc.scalar.copy(
                    xpad_v[:, img, 0:PAD, :],
                    zrow.rearrange("c (h w) -> c h w", h=PAD),
                )
                nc.scalar.copy(
                    xpad_v[:, img, HP - PAD : HP, :],
                    zrow.rearrange("c (h w) -> c h w", h=PAD),
                )
                nc.scalar.copy(
                    xpad_v[:, img, PAD : PAD + H, 0:PAD],
                    zrow[:, : H * PAD].rearrange("c (h w) -> c h w", h=H),
                )
                nc.scalar.copy(
                    xpad_v[:, img, PAD : PAD + H, WP - PAD : WP],
                    zrow[:, : H * PAD].rearrange("c (h w) -> c h w", h=H),
                )
            CHUNK_ROWS = 16
            for img in range(IMGS_PER_CORE):
                for c0 in range(0, H, CHUNK_ROWS):
                    st = stage.tile([CIN, CHUNK_ROWS * W], F32, name="xstage")
                    nc.sync.dma_start(
                        out=st,
                        in_=x_in[img, :, c0 : c0 + CHUNK_ROWS, :].rearrange(
                            "c h w -> c (h w)"
                        ),
                    )
                    nc.scalar.copy(
                        xpad_v[:, img, PAD + c0 : PAD + c0 + CHUNK_ROWS, PAD : PAD + W],
                        st.rearrange("c (h w) -> c h w", h=CHUNK_ROWS),
                    )

            tc.tile_set_cur_wait(0, enable=False)
            tc.cur_wait_ts = None
            # ---- convolution (rotation 0 last so its V-slice has time) ----
            out_v = out[:, :, :, :]  # [img, rco, y, x]
            for r in (1, 2, 3, 0):
                if r == 0:
                    # rotation 0 = the (rounded) original weight: pure
                    # reshuffle [ci,(co,q)] -> [ci,(p,co)], emitted here so
                    # the DVE runs it under the r=1..3 conv stream
                    nc.vector.tensor_copy(
                        vbuf_v[:, 0, :, :],
                        wnat_v[:, :, 0:TAPS].transpose([0, 2, 1]),
                    )
                for img in range(IMGS_PER_CORE):
                    for t in range(N_TILES):
                        acc = ps.tile(
                            [COUT, ROWS_PER_TILE * W],
                            mybir.dt.float32,
                            name="acc",
                            tag="conv",
                            bufs=2,
                        )
                        y0 = t * ROWS_PER_TILE
                        for p in range(TAPS):
                            dy, dx = p // KS, p % KS
                            rhs = xpad_v[
                                :, img, y0 + dy : y0 + dy + ROWS_PER_TILE,
                                dx : dx + W,
                            ]
                            nc.tensor.matmul(
                                acc,
                                vbuf_v[:, r, p, :],
                                rhs,
                                start=(p == 0),
                                stop=(p == TAPS - 1),
                            )
                        ot = outp.tile([COUT, ROWS_PER_TILE * W], F32, name="ot")
                        nc.vector.tensor_copy(ot, acc)
                        nc.sync.dma_start(
                            out=out_v[
                                img,
                                r * COUT : (r + 1) * COUT,
                                y0 : y0 + ROWS_PER_TILE,
                                :,
                            ].rearrange("o h w -> o (h w)"),
                            in_=ot,
                        )

    nc.finalize()
    return nc


_NC_CACHE = None


def _get_nc():
    global _NC_CACHE
    if _NC_CACHE is None:
        _NC_CACHE = _build()
    return _NC_CACHE


def kernel(
    input: np.ndarray,
    weight: np.ndarray,
    _trace: bool = False,
    _trace_cores=None,
    _result_holder: dict | None = None,
) -> np.ndarray:
    input = np.ascontiguousarray(np.asarray(input, dtype=np.float32))
    weight = np.ascontiguousarray(np.asarray(weight, dtype=np.float32))
    assert input.shape == (B, CIN, H, W), input.shape
    assert weight.shape == (COUT, CIN, KS, KS), weight.shape

    mcat = _mcat()
    ident = np.eye(CIN, dtype=np.float32)

    nc = _get_nc()
    in_maps = []
    for core in range(N_CORES):
        in_maps.append(
            {
                "x": input[core * IMGS_PER_CORE : (core + 1) * IMGS_PER_CORE],
                "w": weight,
                "mcat": mcat,
                "ident": ident,
            }
        )

    kwargs = {}
    if _trace:
        kwargs["trace"] = True
        if _trace_cores is not None:
            kwargs["trace_cores"] = _trace_cores

    res = run_bass_kernel_spmd(nc, in_maps, list(range(N_CORES)), **kwargs)
    if _result_holder is not None:
        _result_holder["res"] = res

    out = np.concatenate([res.results[c]["out"] for c in range(N_CORES)], axis=0)
    return out
